# revision 10
# baseline (speedup 1.0000x reference)
"""Trainium2 Bass kernel for nn_GSubgroupKernel (SO(2) group-kernel expansion).

Math: out[oc, i, ic, j] = (1-f[i,j]) * W[oc,ic,i0[i,j]] + f[i,j] * W[oc,ic,i1[i,j]]
which factors as a K=16 matmul  res[s=(i,j), m=(ic,oc)] = sum_k A[k,s] * Wt[k,m]
where A is the (2-nonzeros-per-column) circular-interpolation matrix built from
the tiny angle inputs, and Wt is weight transposed to [K, ic*oc].

Sharding: out_channels split across 8 NeuronCores (48 each); A is replicated.
Precision: fp32 operands are split hi/lo into bf16 and the product is formed as
A_hi@W_hi + A_lo@W_hi + A_hi@W_lo accumulated in fp32 PSUM (error ~1e-5 rel,
full bf16 matmul rate). Each core writes a [256, 18432] slab; the host
reassembles the [384,16,384,16,1,1] output.
"""

import os
import sys

import numpy as np

for _p in ("/opt/trn_rl_repo",):
    if os.path.isdir(_p) and _p not in sys.path:
        sys.path.append(_p)

OUT_C = 384
IN_C = 384
K = 16
NUM_IN = 16
NUM_OUT = 16
N_CORES = 8
OC_PER = OUT_C // N_CORES            # 48 out-channels per core
M = IN_C * OC_PER                    # 18432 matmul columns per core
S = NUM_IN * NUM_OUT                 # 256 interpolation points
TWO_PI = 2.0 * np.pi

CHUNK = 512                          # matmul free dim (one PSUM bank, fp32)
GRP = 9                              # chunks per staging buffer / output DMA
NGRP = M // (CHUNK * GRP)            # 4 column groups

MODE = os.environ.get("GSK_MODE", "bf16x3")   # "bf16x3" | "f32r"

_CACHE = {}


KSTACK = 3 * K                       # hi/lo-split terms stacked along contraction


def _build_nc_bf16x3():
    import concourse.bacc as bacc
    import concourse.mybir as mybir
    from concourse import tile

    f32 = mybir.dt.float32
    bf16 = mybir.dt.bfloat16

    nc = bacc.Bacc("TRN2", target_bir_lowering=False, debug=False)
    wt = nc.dram_tensor("wt", [KSTACK, M], bf16, kind="ExternalInput").ap()
    a = nc.dram_tensor("a", [KSTACK, S], bf16, kind="ExternalInput").ap()
    res = nc.dram_tensor("res", [S, M], f32, kind="ExternalOutput").ap()

    gw = CHUNK * GRP                 # 4608 columns per group
    SUB = 3                          # chunks per output DMA piece
    with tile.TileContext(nc) as tc:
        with tc.tile_pool(name="const", bufs=1) as cpool, \
             tc.tile_pool(name="win", bufs=2) as wpool, \
             tc.tile_pool(name="stage", bufs=8) as spool, \
             tc.tile_pool(name="ps", bufs=8, space="PSUM") as ppool:
            a_sb = cpool.tile([KSTACK, S], bf16)
            # input DMAs ride the ACT HWDGE ring; output DMAs ride SP's —
            # keeps weight prefetch out of the output ring's FIFO
            nc.scalar.dma_start(out=a_sb[:, :], in_=a)
            for g in range(NGRP):
                w_sb = wpool.tile([KSTACK, gw], bf16)
                if g == 0:
                    # split the first group so the first matmuls unblock early
                    for wp in range(GRP // SUB):
                        ws = slice(wp * SUB * CHUNK, (wp + 1) * SUB * CHUNK)
                        nc.scalar.dma_start(
                            out=w_sb[:, ws],
                            in_=wt[:, g * gw + ws.start:g * gw + ws.stop],
                        )
                else:
                    nc.scalar.dma_start(
                        out=w_sb[:, :], in_=wt[:, g * gw:(g + 1) * gw]
                    )
                for half in range(2):
                    hs = slice(half * 128, (half + 1) * 128)
                    stage = spool.tile([128, gw], f32)
                    for ci in range(GRP):
                        cs = slice(ci * CHUNK, (ci + 1) * CHUNK)
                        ps = ppool.tile([128, CHUNK], f32)
                        nc.tensor.matmul(
                            ps[:, :], lhsT=a_sb[:, hs], rhs=w_sb[:, cs],
                            start=True, stop=True,
                        )
                        # split PSUM->SBUF copies across DVE and ACT
                        if ci % 2 == 0:
                            nc.vector.tensor_copy(out=stage[:, cs], in_=ps[:, :])
                        else:
                            nc.scalar.copy(out=stage[:, cs], in_=ps[:, :])
                        if ci % SUB == SUB - 1:
                            ss = slice((ci - SUB + 1) * CHUNK, (ci + 1) * CHUNK)
                            nc.sync.dma_start(
                                out=res[hs, g * gw + ss.start:g * gw + ss.stop],
                                in_=stage[:, ss],
                            )
    nc.compile()
    return nc


def _build_nc_f32r():
    import concourse.bacc as bacc
    import concourse.mybir as mybir
    from concourse import tile

    f32 = mybir.dt.float32
    f32r = mybir.dt.float32r

    nc = bacc.Bacc("TRN2", target_bir_lowering=False, debug=False)
    wt = nc.dram_tensor("wt", [K, M], f32r, kind="ExternalInput").ap()
    a = nc.dram_tensor("a", [K, S], f32r, kind="ExternalInput").ap()
    res = nc.dram_tensor("res", [S, M], f32, kind="ExternalOutput").ap()

    gw = CHUNK * GRP
    with tile.TileContext(nc) as tc:
        with tc.tile_pool(name="const", bufs=1) as cpool, \
             tc.tile_pool(name="win", bufs=2) as wpool, \
             tc.tile_pool(name="stage", bufs=4) as spool, \
             tc.tile_pool(name="ps", bufs=8, space="PSUM") as ppool:
            a_sb = cpool.tile([K, S], f32r)
            nc.sync.dma_start(out=a_sb[:, :], in_=a)
            for g in range(NGRP):
                w_sb = wpool.tile([K, gw], f32r)
                nc.sync.dma_start(out=w_sb[:, :], in_=wt[:, g * gw:(g + 1) * gw])
                for half in range(2):
                    stage = spool.tile([128, gw], f32)
                    for ci in range(GRP):
                        ps = ppool.tile([128, CHUNK], f32)
                        nc.tensor.matmul(
                            ps[:, :],
                            lhsT=a_sb[:, half * 128:(half + 1) * 128],
                            rhs=w_sb[:, ci * CHUNK:(ci + 1) * CHUNK],
                            start=True,
                            stop=True,
                        )
                        nc.vector.tensor_copy(
                            out=stage[:, ci * CHUNK:(ci + 1) * CHUNK], in_=ps[:, :]
                        )
                    nc.sync.dma_start(
                        out=res[half * 128:(half + 1) * 128, g * gw:(g + 1) * gw],
                        in_=stage[:, :],
                    )
    nc.compile()
    return nc


def _interp_matrix(in_H, out_H):
    """A[k, s] for s=(i,j): (1-frac) at k=i0, frac at k=i1 — fp32, mirroring
    the reference's circular linear interpolation on the uniform K-grid."""
    inH = np.asarray(in_H, dtype=np.float32).reshape(-1)
    outH = np.asarray(out_H, dtype=np.float32).reshape(-1)
    prod = np.mod(inH[:, None] - outH[None, :], np.float32(TWO_PI))
    coords = prod.reshape(-1).astype(np.float32)
    pos = coords / np.float32(TWO_PI / K)
    base = np.floor(pos)
    i0 = np.mod(base.astype(np.int32), K)
    i1 = np.mod(i0 + 1, K)
    frac = (pos - base).astype(np.float32)
    A = np.zeros((K, S), dtype=np.float32)
    cols = np.arange(S)
    np.add.at(A, (i0, cols), np.float32(1.0) - frac)
    np.add.at(A, (i1, cols), frac)
    return A


def _hi_lo(x):
    import ml_dtypes

    hi = x.astype(ml_dtypes.bfloat16)
    lo = (x - hi.astype(np.float32)).astype(ml_dtypes.bfloat16)
    return hi, lo


def kernel(in_H, out_H, weight, grid_H):
    from concourse.bass_utils import run_bass_kernel_spmd

    weight = np.asarray(weight, dtype=np.float32)
    A = _interp_matrix(in_H, out_H)

    in_maps = []
    if MODE == "bf16x3":
        a_hi, a_lo = _hi_lo(A)
        a_stack = np.concatenate([a_hi, a_lo, a_hi], axis=0)       # [48, S]
        for c in range(N_CORES):
            w_c = weight[c * OC_PER:(c + 1) * OC_PER]      # [48, 384, 16]
            wt_c = np.ascontiguousarray(w_c.transpose(2, 1, 0)).reshape(K, M)
            wt_hi, wt_lo = _hi_lo(wt_c)
            wt_stack = np.concatenate([wt_hi, wt_hi, wt_lo], axis=0)  # [48, M]
            in_maps.append({"wt": wt_stack, "a": a_stack})
    else:
        for c in range(N_CORES):
            w_c = weight[c * OC_PER:(c + 1) * OC_PER]
            wt_c = np.ascontiguousarray(w_c.transpose(2, 1, 0)).reshape(K, M)
            in_maps.append({"wt": wt_c, "a": A})

    key = "nc_" + MODE
    if key not in _CACHE:
        _CACHE[key] = (
            _build_nc_bf16x3() if MODE == "bf16x3" else _build_nc_f32r()
        )
    r = run_bass_kernel_spmd(_CACHE[key], in_maps, list(range(N_CORES)))
    _CACHE["last_result"] = r
    if r.exec_time_ns is not None:
        print(f"HW exec time: {r.exec_time_ns} ns")

    res_all = np.stack([r.results[c]["res"] for c in range(N_CORES)])
    out = res_all.reshape(N_CORES, NUM_IN, NUM_OUT, IN_C, OC_PER)
    out = np.ascontiguousarray(out.transpose(0, 4, 1, 3, 2))
    return out.reshape(OUT_C, NUM_IN, IN_C, NUM_OUT, 1, 1)


# revision 11
# speedup vs baseline: 1.1228x; 1.1228x over previous
"""Trainium2 Bass kernel for nn_GSubgroupKernel (SO(2) group-kernel expansion).

Math: out[oc, i, ic, j] = (1-f[i,j]) * W[oc,ic,i0[i,j]] + f[i,j] * W[oc,ic,i1[i,j]]
which factors as a K=16 matmul  res[s=(i,j), m=(ic,oc)] = sum_k A[k,s] * Wt[k,m]
where A is the (2-nonzeros-per-column) circular-interpolation matrix built from
the tiny angle inputs, and Wt is weight transposed to [K, ic*oc].

Sharding: out_channels split across 8 NeuronCores (48 each); A is replicated.
Precision: fp32 operands are split hi/lo into bf16 and the product is formed as
A_hi@W_hi + A_lo@W_hi + A_hi@W_lo accumulated in fp32 PSUM (error ~1e-5 rel,
full bf16 matmul rate). Each core writes a [256, 18432] slab; the host
reassembles the [384,16,384,16,1,1] output.
"""

import os
import sys

import numpy as np

for _p in ("/opt/trn_rl_repo",):
    if os.path.isdir(_p) and _p not in sys.path:
        sys.path.append(_p)

OUT_C = 384
IN_C = 384
K = 16
NUM_IN = 16
NUM_OUT = 16
N_CORES = 8
OC_PER = OUT_C // N_CORES            # 48 out-channels per core
M = IN_C * OC_PER                    # 18432 matmul columns per core
S = NUM_IN * NUM_OUT                 # 256 interpolation points
TWO_PI = 2.0 * np.pi

CHUNK = 512                          # matmul free dim (one PSUM bank, fp32)
GRP = 9                              # chunks per staging buffer / output DMA
NGRP = M // (CHUNK * GRP)            # 4 column groups

MODE = os.environ.get("GSK_MODE", "bf16x3")   # "bf16x3" | "f32r"

_CACHE = {}


KSTACK = 3 * K                       # hi/lo-split terms stacked along contraction


def _build_nc_bf16x3():
    import concourse.bacc as bacc
    import concourse.mybir as mybir
    from concourse import tile

    f32 = mybir.dt.float32
    bf16 = mybir.dt.bfloat16

    nc = bacc.Bacc("TRN2", target_bir_lowering=False, debug=False)
    wt = nc.dram_tensor("wt", [KSTACK, M], bf16, kind="ExternalInput").ap()
    a = nc.dram_tensor("a", [KSTACK, S], bf16, kind="ExternalInput").ap()
    res = nc.dram_tensor("res", [S, M], f32, kind="ExternalOutput").ap()

    gw = CHUNK * GRP                 # 4608 columns per group
    SUB = 3                          # chunks per output DMA piece
    with tile.TileContext(nc) as tc:
        with tc.tile_pool(name="const", bufs=1) as cpool, \
             tc.tile_pool(name="win", bufs=2) as wpool, \
             tc.tile_pool(name="stage", bufs=6) as spool, \
             tc.tile_pool(name="ps", bufs=8, space="PSUM") as ppool:
            a_sb = cpool.tile([KSTACK, S], bf16)
            # input DMAs ride the ACT HWDGE ring; output DMAs ride SP's —
            # keeps weight prefetch out of the output ring's FIFO
            nc.scalar.dma_start(out=a_sb[:, :], in_=a)
            for g in range(NGRP):
                w_sb = wpool.tile([KSTACK, gw], bf16)
                if g == 0:
                    # split the first group so the first matmuls unblock early
                    for wp in range(GRP // SUB):
                        ws = slice(wp * SUB * CHUNK, (wp + 1) * SUB * CHUNK)
                        nc.scalar.dma_start(
                            out=w_sb[:, ws],
                            in_=wt[:, g * gw + ws.start:g * gw + ws.stop],
                        )
                else:
                    nc.scalar.dma_start(
                        out=w_sb[:, :], in_=wt[:, g * gw:(g + 1) * gw]
                    )
                for half in range(2):
                    hs = slice(half * 128, (half + 1) * 128)
                    stage = spool.tile([128, gw], f32)
                    for ci in range(GRP):
                        cs = slice(ci * CHUNK, (ci + 1) * CHUNK)
                        ps = ppool.tile([128, CHUNK], f32)
                        nc.tensor.matmul(
                            ps[:, :], lhsT=a_sb[:, hs], rhs=w_sb[:, cs],
                            start=True, stop=True,
                        )
                        # split PSUM->SBUF copies across DVE and ACT
                        if ci % 2 == 0:
                            nc.vector.tensor_copy(out=stage[:, cs], in_=ps[:, :])
                        else:
                            nc.scalar.copy(out=stage[:, cs], in_=ps[:, :])
                        if ci % SUB == SUB - 1:
                            ss = slice((ci - SUB + 1) * CHUNK, (ci + 1) * CHUNK)
                            nc.sync.dma_start(
                                out=res[hs, g * gw + ss.start:g * gw + ss.stop],
                                in_=stage[:, ss],
                            )
    nc.compile()
    return nc


def _build_nc_f32r():
    import concourse.bacc as bacc
    import concourse.mybir as mybir
    from concourse import tile

    f32 = mybir.dt.float32
    f32r = mybir.dt.float32r

    nc = bacc.Bacc("TRN2", target_bir_lowering=False, debug=False)
    wt = nc.dram_tensor("wt", [K, M], f32r, kind="ExternalInput").ap()
    a = nc.dram_tensor("a", [K, S], f32r, kind="ExternalInput").ap()
    res = nc.dram_tensor("res", [S, M], f32, kind="ExternalOutput").ap()

    gw = CHUNK * GRP
    with tile.TileContext(nc) as tc:
        with tc.tile_pool(name="const", bufs=1) as cpool, \
             tc.tile_pool(name="win", bufs=2) as wpool, \
             tc.tile_pool(name="stage", bufs=4) as spool, \
             tc.tile_pool(name="ps", bufs=8, space="PSUM") as ppool:
            a_sb = cpool.tile([K, S], f32r)
            nc.sync.dma_start(out=a_sb[:, :], in_=a)
            for g in range(NGRP):
                w_sb = wpool.tile([K, gw], f32r)
                nc.sync.dma_start(out=w_sb[:, :], in_=wt[:, g * gw:(g + 1) * gw])
                for half in range(2):
                    stage = spool.tile([128, gw], f32)
                    for ci in range(GRP):
                        ps = ppool.tile([128, CHUNK], f32)
                        nc.tensor.matmul(
                            ps[:, :],
                            lhsT=a_sb[:, half * 128:(half + 1) * 128],
                            rhs=w_sb[:, ci * CHUNK:(ci + 1) * CHUNK],
                            start=True,
                            stop=True,
                        )
                        nc.vector.tensor_copy(
                            out=stage[:, ci * CHUNK:(ci + 1) * CHUNK], in_=ps[:, :]
                        )
                    nc.sync.dma_start(
                        out=res[half * 128:(half + 1) * 128, g * gw:(g + 1) * gw],
                        in_=stage[:, :],
                    )
    nc.compile()
    return nc


def _interp_matrix(in_H, out_H):
    """A[k, s] for s=(i,j): (1-frac) at k=i0, frac at k=i1 — fp32, mirroring
    the reference's circular linear interpolation on the uniform K-grid."""
    inH = np.asarray(in_H, dtype=np.float32).reshape(-1)
    outH = np.asarray(out_H, dtype=np.float32).reshape(-1)
    prod = np.mod(inH[:, None] - outH[None, :], np.float32(TWO_PI))
    coords = prod.reshape(-1).astype(np.float32)
    pos = coords / np.float32(TWO_PI / K)
    base = np.floor(pos)
    i0 = np.mod(base.astype(np.int32), K)
    i1 = np.mod(i0 + 1, K)
    frac = (pos - base).astype(np.float32)
    A = np.zeros((K, S), dtype=np.float32)
    cols = np.arange(S)
    np.add.at(A, (i0, cols), np.float32(1.0) - frac)
    np.add.at(A, (i1, cols), frac)
    return A


def _hi_lo(x):
    import ml_dtypes

    hi = x.astype(ml_dtypes.bfloat16)
    lo = (x - hi.astype(np.float32)).astype(ml_dtypes.bfloat16)
    return hi, lo


def kernel(in_H, out_H, weight, grid_H):
    from concourse.bass_utils import run_bass_kernel_spmd

    weight = np.asarray(weight, dtype=np.float32)
    A = _interp_matrix(in_H, out_H)

    in_maps = []
    if MODE == "bf16x3":
        a_hi, a_lo = _hi_lo(A)
        a_stack = np.concatenate([a_hi, a_lo, a_hi], axis=0)       # [48, S]
        for c in range(N_CORES):
            w_c = weight[c * OC_PER:(c + 1) * OC_PER]      # [48, 384, 16]
            wt_c = np.ascontiguousarray(w_c.transpose(2, 1, 0)).reshape(K, M)
            wt_hi, wt_lo = _hi_lo(wt_c)
            wt_stack = np.concatenate([wt_hi, wt_hi, wt_lo], axis=0)  # [48, M]
            in_maps.append({"wt": wt_stack, "a": a_stack})
    else:
        for c in range(N_CORES):
            w_c = weight[c * OC_PER:(c + 1) * OC_PER]
            wt_c = np.ascontiguousarray(w_c.transpose(2, 1, 0)).reshape(K, M)
            in_maps.append({"wt": wt_c, "a": A})

    key = "nc_" + MODE
    if key not in _CACHE:
        _CACHE[key] = (
            _build_nc_bf16x3() if MODE == "bf16x3" else _build_nc_f32r()
        )
    r = run_bass_kernel_spmd(_CACHE[key], in_maps, list(range(N_CORES)))
    _CACHE["last_result"] = r
    if r.exec_time_ns is not None:
        print(f"HW exec time: {r.exec_time_ns} ns")

    res_all = np.stack([r.results[c]["res"] for c in range(N_CORES)])
    out = res_all.reshape(N_CORES, NUM_IN, NUM_OUT, IN_C, OC_PER)
    out = np.ascontiguousarray(out.transpose(0, 4, 1, 3, 2))
    return out.reshape(OUT_C, NUM_IN, IN_C, NUM_OUT, 1, 1)
